# revision 20
# baseline (speedup 1.0000x reference)
"""Causal self-attention on 8 TRN2 NeuronCores — bf16 tensor-parallel version.

Problem: x[4, 2048, 1024], w_qkv[3072, 1024], w_proj[1024, 1024],
16 heads x 64 dims, causal softmax attention, output [4, 2048, 1024].

Sharding: core c handles (batch b = c//2, head-group hg = c%2).
Each head-group = 8 heads = 512 channels. Tensor-parallel over heads:
each core computes a *partial* projection output [2048, 1024] in bf16;
the host sums the two head-group partials per batch in fp32.

All matmuls are bf16 (rel-err budget 2e-2; bf16 keeps us ~4e-3):
  Phase A:  QT = Wq @ X^T   [512, 2048]   (head-pair channels on partitions)
            KT = Wk @ X^T   [512, 2048]
            V  = X @ Wv^T   [2048, 512]   (+ ones column per head)
            emitted as 8-matmul groups; all but the first ib=0 slices are
            drained one-per-j inside the attention loops so PE fills the
            ACT-paced softmax era and ACT starts exp'ing early.
  Attention, qi-major: per 512-query block qi, per head-pair hp, per
  128-key tile j:
            ST pair: two K=64 row-group-concurrent matmuls -> one
              [128, 1024] 2-bank PSUM tile (head s at cols 512s..)
            diagonal tiles truncated to valid queries (n0 = 128*(j-4qi));
              the leading 128x128 triangle masked by accumulating
              (-1e5*I) @ tri  (one extra N=128 matmul per head)
            PT pair = exp(0.125 * ST) in ONE activation over both heads
            YT_s += [V_h | 1]^T @ PT_s  accumulated over j  ([65, 512]
              PSUM; row 64 = softmax denominators)
  Normalize (batched per qi): sc = copy(YT) per head (frees PSUM);
            DMA the 8 denominator rows into dq[8, 512]; ONE reciprocal;
            per hp: rps = e8^T @ r broadcasts r over the 64 dims of each
            head; ytu = sc * rps (bf16).  All of this is deferred into
            the next query block's j-loops to keep the PE FIFO moving.
  Proj:     out[it*128.., nb*512..] partial = sum_pc ytu_pc^T-contracted
            with w_proj slice; bf16 out via DMA.  Also deferred.
"""

import numpy as np
from collections import deque
from contextlib import ExitStack

import ml_dtypes

import concourse.bass as bass
import concourse.tile as tile
from concourse import bacc, mybir
from concourse.bass_utils import run_bass_kernel_spmd

B, T, C, H, D = 4, 2048, 1024, 16, 64
HG = 2                 # head groups (tensor-parallel ways)
CG = 512               # channels per head group
P = 128
NQI = T // 512         # 4 query blocks
NJT = T // P           # 16 key tiles
NEG = -1.0e5           # causal mask additive constant (exp(0.125*NEG) == 0)
F32 = mybir.dt.float32
BF16 = mybir.dt.bfloat16
BF16_NP = ml_dtypes.bfloat16

_CACHE = {}


def _build_core_program():
    nc = bacc.Bacc("TRN2", target_bir_lowering=False, debug=False, num_devices=8)
    xt = nc.dram_tensor("xt", [C, T], BF16, kind="ExternalInput").ap()
    wqkvt = nc.dram_tensor("wqkvt", [C, 3 * CG], BF16, kind="ExternalInput").ap()
    wpt = nc.dram_tensor("wpt", [CG, C], BF16, kind="ExternalInput").ap()
    out = nc.dram_tensor("out", [T, C], BF16, kind="ExternalOutput").ap()

    with tile.TileContext(nc) as tc:
        with ExitStack() as ctx:
            _attention(ctx, tc, xt, wqkvt, wpt, out)
    nc.compile()
    return nc


def _attention(ctx, tc, xt, wqkvt, wpt, out):
    nc = tc.nc

    # ---------------- persistent SBUF ----------------
    persist = ctx.enter_context(tc.tile_pool(name="persist", bufs=1))
    qt = persist.tile([P, 4, T], BF16, tag="qt")       # QT[hp*128+p, t] at [p, hp, t]
    kt = persist.tile([P, 4, T], BF16, tag="kt")
    v = persist.tile([P, NJT, 8 * 65], BF16, tag="v")  # [V_h | 1] per key tile
    ytu = persist.tile([P, 4, T], BF16, tag="ytu")     # normalized YT (bf16)
    wpt_sb = persist.tile([P, 4, C], BF16, tag="wpt")

    # ---------------- constants ----------------
    consts = ctx.enter_context(tc.tile_pool(name="consts", bufs=1))
    with ExitStack() as cstage:
        stage = cstage.enter_context(tc.tile_pool(name="cstage", bufs=2))
        ns = stage.tile([P, P], F32, tag="cst", name="negIs")
        nc.gpsimd.memset(ns, 0.0)
        # keep 0 where (q - p) != 0, fill NEG on the diagonal -> NEG * I
        nc.gpsimd.affine_select(
            out=ns, in_=ns, compare_op=mybir.AluOpType.not_equal, fill=NEG,
            base=0, pattern=[[-1, P]], channel_multiplier=1,
        )
        neg_i = consts.tile([P, P], BF16, tag="negI")
        nc.vector.tensor_copy(neg_i, ns)

        ts_ = stage.tile([P, P], F32, tag="cst", name="tris")
        nc.gpsimd.memset(ts_, 0.0)
        # keep 0 where (q - p) >= 0 (valid), fill 1 where q < p (masked)
        nc.gpsimd.affine_select(
            out=ts_, in_=ts_, compare_op=mybir.AluOpType.is_ge, fill=1.0,
            base=0, pattern=[[1, P]], channel_multiplier=-1,
        )
        tri = consts.tile([P, P], BF16, tag="tri")
        nc.vector.tensor_copy(tri, ts_)

        # e8[pc][p, c] = 1 iff p == 2*pc + c//64 : broadcasts r rows onto
        # the 64 channel-partitions of each head of pair pc.
        es = stage.tile([8, 512], F32, tag="cst2", name="e8s")
        nc.gpsimd.memset(es, 0.0)
        e4d = es.rearrange("p (c a b) -> p c a b", a=2, b=64)
        nc.gpsimd.affine_select(
            out=e4d, in_=e4d, compare_op=mybir.AluOpType.not_equal, fill=1.0,
            base=0, pattern=[[-2, 4], [-1, 2], [0, 64]], channel_multiplier=1,
        )
        e8 = []
        for pc in range(4):
            t = consts.tile([8, P], BF16, tag=f"e8_{pc}", name=f"e8_{pc}")
            nc.vector.tensor_copy(t, es[:, pc * P:(pc + 1) * P])
            e8.append(t)
        # e2[s, c] = 1 iff c//64 == s (2-row variant for the hp==3 chunk)
        e2s = stage.tile([2, P], F32, tag="cst3", name="e2s")
        nc.gpsimd.memset(e2s, 0.0)
        e2d = e2s.rearrange("p (a b) -> p a b", b=64)
        nc.gpsimd.affine_select(
            out=e2d, in_=e2d, compare_op=mybir.AluOpType.not_equal, fill=1.0,
            base=0, pattern=[[-1, 2], [0, 64]], channel_multiplier=1,
        )
        e2 = consts.tile([2, P], BF16, tag="e2")
        nc.vector.tensor_copy(e2, e2s)

    v4 = v.rearrange("p j (h f) -> p j h f", f=65)
    nc.gpsimd.memset(v4[:, :, :, 64:65], 1.0)   # ones column of each head

    # ---------------- input DMA ----------------
    # One DMA per chunk (contiguous 2D transfers), alternating between the
    # two HWDGE rings (sync + scalar) so transfers overlap: HWDGE DMAs are
    # FIFO per issuing engine.
    xw = ctx.enter_context(tc.tile_pool(name="xw", bufs=8))
    xs, ws = [], []
    for cc in range(8):
        ring_x = nc.sync if cc % 2 == 0 else nc.scalar
        ring_w = nc.scalar if cc % 2 == 0 else nc.sync
        xc = xw.tile([P, T], BF16, tag="x", name=f"x{cc}")
        ring_x.dma_start(xc, xt[cc * P:(cc + 1) * P, :])
        wc = xw.tile([P, 3 * CG], BF16, tag="w", name=f"w{cc}")
        ring_w.dma_start(wc, wqkvt[cc * P:(cc + 1) * P, :])
        xs.append(xc)
        ws.append(wc)
    nc.scalar.dma_start(wpt_sb, wpt.rearrange("(pc p) n -> p pc n", p=P))

    # ---------------- phase A group emitters ----------------
    def kq_group(pool, tag, dst, hp, ib, wbase):
        def emit():
            ps = pool.tile([P, 512], F32, tag=tag)
            for cc in range(8):
                nc.tensor.matmul(
                    ps, ws[cc][:, wbase + hp * P: wbase + hp * P + P],
                    xs[cc][:, ib * 512:(ib + 1) * 512],
                    start=(cc == 0), stop=(cc == 7),
                )
            nc.vector.tensor_copy(dst[:, hp, ib * 512:(ib + 1) * 512], ps)
        return emit

    def v_group(pool, tag, it):
        def emit():
            ps = pool.tile([P, 512], F32, tag=tag)
            for cc in range(8):
                nc.tensor.matmul(
                    ps, xs[cc][:, it * P:(it + 1) * P],
                    ws[cc][:, 2 * CG:3 * CG],
                    start=(cc == 0), stop=(cc == 7),
                )
            psv = ps.rearrange("p (h e) -> p h e", e=64)
            nc.vector.tensor_copy(v4[:, it, :, 0:64], psv)
        return emit

    # ---------------- upfront phase A (own 2-bank PSUM pool) ----------------
    # only what the very first j-iteration needs; the rest drains as
    # `pending` inside the attention loops.
    with ExitStack() as upctx:
        up_ps = upctx.enter_context(
            tc.tile_pool(name="up_ps", bufs=2, space="PSUM")
        )
        kq_group(up_ps, "up", kt, 0, 0, CG)()
        kq_group(up_ps, "up", qt, 0, 0, 0)()

    # ---------------- attention-era pools (8 PSUM banks exactly) ----------
    st_ps = ctx.enter_context(tc.tile_pool(name="st_ps", bufs=2, space="PSUM"))
    yt_ps = ctx.enter_context(tc.tile_pool(name="yt_ps", bufs=1, space="PSUM"))
    aux_ps = ctx.enter_context(tc.tile_pool(name="aux_ps", bufs=2, space="PSUM"))
    pt_pool = ctx.enter_context(tc.tile_pool(name="pt", bufs=4))
    sc_pool = ctx.enter_context(tc.tile_pool(name="sc", bufs=2))
    dq_pool = ctx.enter_context(tc.tile_pool(name="dq", bufs=2))
    r8_pool = ctx.enter_context(tc.tile_pool(name="r8", bufs=2))
    osb_pool = ctx.enter_context(tc.tile_pool(name="osb", bufs=2))

    # remaining phase A work, drained a few groups per j inside attention.
    # Order matters: query block qi needs KT/QT ib<=qi and V tiles <=4qi+3;
    # head pair hp of qi=0 needs its KT/QT ib=0 before its j-loop starts.
    pending = deque()
    for it in range(4):
        pending.append(v_group(aux_ps, "aux", it))
    for hp in range(1, 4):
        pending.append(kq_group(aux_ps, "aux", kt, hp, 0, CG))
        pending.append(kq_group(aux_ps, "aux", qt, hp, 0, 0))
    for ib in range(1, 4):
        for it in range(4 * ib, 4 * ib + 4):
            pending.append(v_group(aux_ps, "aux", it))
        for hp in range(4):
            pending.append(kq_group(aux_ps, "aux", kt, hp, ib, CG))
            pending.append(kq_group(aux_ps, "aux", qt, hp, ib, 0))

    # ---------------- attention ----------------
    deferred = deque()    # normalize + proj closures, flushed a few per j
    held = []             # late qi=2 proj groups: emitted inside qi=3/hp=3
                          # to keep PE fed (and the HAM clock warm) while
                          # the final normalize chain runs
    for qi in range(NQI):
        njt = 4 * qi + 4
        dq = dq_pool.tile([6, 512], F32, tag="dq")      # denoms, hp 0-2
        dq2 = dq_pool.tile([2, 512], F32, tag="dq2")    # denoms, hp 3
        scs = {}
        for hp in range(4):
            yts = [yt_ps.tile([65, 512], F32, tag=f"yt{s}", name=f"yt{s}")
                   for s in range(2)]
            yt_pend = None
            for j in range(njt):
                o = j - 4 * qi          # >= 0 on diagonal tiles
                n0 = 128 * o if o >= 0 else 0
                stp = st_ps.tile([P, 1024], F32, tag="st")
                for s in range(2):
                    r0 = 64 * s
                    nc.tensor.matmul(
                        stp[:, s * 512 + n0: (s + 1) * 512],
                        kt[r0:r0 + 64, hp, j * P:(j + 1) * P],
                        qt[r0:r0 + 64, hp, qi * 512 + n0: (qi + 1) * 512],
                        start=True, stop=(o < 0),
                    )
                    if o >= 0:
                        nc.tensor.matmul(
                            stp[:, s * 512 + n0: s * 512 + n0 + P],
                            neg_i, tri, start=False, stop=True,
                        )
                ptp = pt_pool.tile([P, 1024], BF16, tag="pt")
                stv = stp.rearrange("p (s q) -> p s q", s=2)[:, :, n0:512]
                ptv = ptp.rearrange("p (s q) -> p s q", s=2)[:, :, n0:512]
                nc.scalar.activation(
                    ptv, stv, mybir.ActivationFunctionType.Exp, scale=0.125
                )
                if deferred:
                    deferred.popleft()()
                quota = 3 if (qi == 0 and hp == 0) else (2 if qi == 0 else 1)
                for _ in range(quota):
                    if pending:
                        pending.popleft()()
                if yt_pend is not None:
                    yt_pend()

                def mk_yt(j=j, n0=n0, ptp=ptp, hp=hp, yts=yts,
                          last=(j == njt - 1)):
                    def e():
                        for s in range(2):
                            h = 2 * hp + s
                            nc.tensor.matmul(
                                yts[s][:, n0:512],
                                v[:, j, h * 65:(h + 1) * 65],
                                ptp[:, s * 512 + n0:(s + 1) * 512],
                                start=(j == 0), stop=last,
                            )
                    return e
                yt_pend = mk_yt()
            yt_pend()
            if qi == 3 and hp == 3:
                for h in held:
                    h()
                held.clear()

            # copy YT out of PSUM (frees the banks), stash denominator rows
            for s in range(2):
                sc = sc_pool.tile([65, 512], F32, tag=f"sc{hp}{s}",
                                  name=f"sc{hp}{s}")
                nc.vector.tensor_copy(sc, yts[s])
                if hp < 3:
                    nc.sync.dma_start(dq[2 * hp + s:2 * hp + s + 1, :],
                                      sc[64:65, :])
                else:
                    nc.sync.dma_start(dq2[s:s + 1, :], sc[64:65, :])
                scs[(hp, s)] = sc

            def mk_norm(hp2, rkey, rhold, qi=qi, scs=scs):
                def e():
                    rps = aux_ps.tile([P, 512], F32, tag="aux")
                    lhsT = e8[hp2][0:6, :] if hp2 < 3 else e2
                    nc.tensor.matmul(rps, lhsT, rhold[rkey],
                                     start=True, stop=True)
                    for s in range(2):
                        nc.vector.tensor_mul(
                            out=ytu[64 * s:64 * s + 64, hp2,
                                    qi * 512:(qi + 1) * 512],
                            in0=scs[(hp2, s)][0:64, :],
                            in1=rps[64 * s:64 * s + 64, :],
                        )
                return e

            if hp == 2:
                # hp 0-2 denominators complete: their normalize can overlap
                # hp3's j-loop (flushed via the deferred pops)
                rhold = {}

                def mk_recip6(dq=dq, rhold=rhold):
                    def e():
                        rf = r8_pool.tile([6, 512], F32, tag="rf")
                        rb = r8_pool.tile([6, 512], BF16, tag="rb")
                        nc.vector.reciprocal(out=rf, in_=dq)
                        nc.vector.tensor_copy(rb, rf)
                        rhold["rb"] = rb
                    return e
                deferred.append(mk_recip6())
                for h2 in range(3):
                    deferred.append(mk_norm(h2, "rb", rhold))
            elif hp == 3:
                rhold2 = {}

                def mk_recip2(dq2=dq2, rhold2=rhold2):
                    def e():
                        rf = r8_pool.tile([2, 512], F32, tag="rf2")
                        rb = r8_pool.tile([2, 512], BF16, tag="rb2")
                        nc.vector.reciprocal(out=rf, in_=dq2)
                        nc.vector.tensor_copy(rb, rf)
                        rhold2["rb2"] = rb
                    return e
                deferred.append(mk_recip2())
                deferred.append(mk_norm(3, "rb2", rhold2))

        for gi, (it, nb) in enumerate(
                (it, nb) for it in range(4 * qi, 4 * qi + 4)
                for nb in range(2)):
            def mk_proj(it=it, nb=nb):
                def e():
                    ops = aux_ps.tile([P, 512], F32, tag="aux")
                    for pc in range(4):
                        nc.tensor.matmul(
                            ops, ytu[:, pc, it * P:(it + 1) * P],
                            wpt_sb[:, pc, nb * 512:(nb + 1) * 512],
                            start=(pc == 0), stop=(pc == 3),
                        )
                    osb = osb_pool.tile([P, 512], BF16, tag="osb")
                    nc.vector.tensor_copy(osb, ops)
                    nc.scalar.dma_start(
                        out[it * P:(it + 1) * P,
                            nb * 512:(nb + 1) * 512], osb
                    )
                return e
            if qi == 2 and (it >= 4 * qi + 2 and nb == 1 or it == 4 * qi + 3 and nb == 0):
                held.append(mk_proj())     # fill the qi=3 tail gap instead
            else:
                deferred.append(mk_proj())

    while deferred:
        deferred.popleft()()
    while pending:
        pending.popleft()()


def _prep_inputs(x, w_qkv, w_proj):
    """Build the 8 per-core input maps (host-side sharding + bf16 casts)."""
    xts = [np.ascontiguousarray(x[b].T).astype(BF16_NP) for b in range(B)]
    wqkvts, wpts = [], []
    for hg in range(HG):
        s = hg * CG
        wq = w_qkv[s:s + CG]
        wk = w_qkv[C + s:C + s + CG]
        wv = w_qkv[2 * C + s:2 * C + s + CG]
        wqkvts.append(
            np.ascontiguousarray(np.concatenate([wq, wk, wv], 0).T).astype(BF16_NP)
        )
        wpts.append(np.ascontiguousarray(w_proj[:, s:s + CG].T).astype(BF16_NP))
    in_maps = []
    for c in range(8):
        b, hg = c // 2, c % 2
        in_maps.append({"xt": xts[b], "wqkvt": wqkvts[hg], "wpt": wpts[hg]})
    return in_maps


def kernel(x, w_qkv, w_proj):
    x = np.asarray(x, dtype=np.float32)
    w_qkv = np.asarray(w_qkv, dtype=np.float32)
    w_proj = np.asarray(w_proj, dtype=np.float32)

    if "nc" not in _CACHE:
        _CACHE["nc"] = _build_core_program()
    nc = _CACHE["nc"]

    in_maps = _prep_inputs(x, w_qkv, w_proj)
    res = run_bass_kernel_spmd(nc, in_maps, core_ids=list(range(8)))
    outs = [r["out"] for r in res.results]
    full = np.empty((B, T, C), dtype=np.float32)
    for b in range(B):
        full[b] = outs[2 * b].astype(np.float32) + outs[2 * b + 1].astype(np.float32)
    return full
